# revision 3
# baseline (speedup 1.0000x reference)
"""BiSPA (bidirectional sparse windowed attention + MLP) Trainium2 kernel.

Full inputs in, full outputs out. Internally shards across 8 NeuronCores:
core c owns output rows Ic = [24c, 24c+24) of the (192, 192, 512) grid.

Key observation: with B == S == 192 and window W == 32,
  - vertical attention for output row i is a complete 192-token sliding-window
    attention over x[i, :, :]                        (needs x rows  Ic)
  - horizontal attention for output row i is a complete 192-token
    sliding-window attention with Q from x[i, :, :] and K/V from x[:, i, :]
    (needs x columns Ic)
so each core needs x[Ic, :, :] and x[:, Ic, :] and NOTHING else -> zero
duplicated projection FLOPs, zero collectives, no halos.

v2 changes vs the first working kernel (tensor-engine bound at ~864us):
  - out-projections folded into MLP1 on the host: F_h = W1h @ Who,
    F_v = W1v @ Wvo, c1 = W1h@h_out_eff + W1v@v_out_eff + b1. Removes
    384 N=384 matmuls + 8 evictions per pair.
  - ctx^T transposes moved off the PE onto the DMA xbar
    (dma_start_transpose), freeing ~65us of TensorE.
  - scores for a head pair land in one 2-bank PSUM tile -> ONE exp
    activation per head pair (batched [128, 2, 224] strided read).
  - softmax normalization: one batched reciprocal per head pair
    ([128,2,2,1] AP over the 4 Z columns), and the 4 scale ops split
    2 on DVE (tensor_scalar_mul) / 2 on ACT (activation scale=AP).
  - emission software-pipelined: QK projections of pair g+1 interleave
    with the attention inner loop of pair g so the PE array duty never
    dips long enough for HAM to re-throttle the clock (354us of the
    baseline ran at 1.2 GHz instead of 2.4).

Numerics: all matmul inputs bf16, fp32 PSUM accumulation, fp32 softmax exp
input, bf16 probs/ctx.  Measured end-to-end rel err vs fp32 reference ~0.6%.
"""

import numpy as np
from contextlib import ExitStack

import concourse.bass as bass
import concourse.mybir as mybir
import concourse.tile as tile
from concourse import bacc
from concourse.bass_utils import run_bass_kernel_spmd
from concourse.tile import add_dep_helper


def _chain(insts):
    """Order matmuls targeting one PSUM bank: a start=True zeroes (marks
    pending-zero) the WHOLE 2KB bank, so each bank must hold exactly one
    accumulation group and the group's matmuls must execute in program order.
    Tile won't order disjoint-region writes by itself."""
    for a, b in zip(insts, insts[1:]):
        add_dep_helper(b.ins, a.ins, sync=False, reason="psum-bank group order")

BF = mybir.dt.bfloat16
F32 = mybir.dt.float32
AF = mybir.ActivationFunctionType
MUL = mybir.AluOpType.mult
NPBF = mybir.dt.np(BF)

E = 512
H = 8
D = 64
W = 32
S = 192
NCORE = 8
RPC = 24          # rows (strips) per core
T = RPC * S       # tokens per core per branch = 4608
NPAIR = RPC // 2  # strip pairs per core


def _band_masks():
    """Score mask, bf16 (128, 448): [TA 128 | TB 96 | pad] x 2 heads.

    TA: rows p = key k in [0,128), cols q in [0,128).
        valid = |k-q| <= W and not (k >= 96 and q >= 96)
    TB: rows p -> key k = 64+p in [64,192), cols q' -> q = 96+q' in [96,192).
        valid = |k-q| <= W and k >= 96
    """
    m = np.zeros((128, 224), np.float32)
    k = np.arange(128)[:, None]
    q = np.arange(128)[None, :]
    ta = (np.abs(k - q) <= W) & ~((k >= 96) & (q >= 96))
    kb = 64 + np.arange(128)[:, None]
    qb = 96 + np.arange(96)[None, :]
    tb = (np.abs(kb - qb) <= W) & (kb >= 96)
    m[:, 0:128] = ta
    m[:, 128:224] = tb
    return np.concatenate([m, m], axis=1).astype(NPBF)


def _build_program(bias_flags):
    """Build the SPMD Bass/Tile program (same program on all 8 cores)."""
    has_vqk_b, has_hq_b, has_hk_b, has_c1, has_b2 = bias_flags

    nc = bacc.Bacc("TRN2", target_bir_lowering=False, debug=False,
                   num_devices=NCORE, num_swdge_queues=4)

    xr_t = nc.dram_tensor("xr_t", [E, T], BF, kind="ExternalInput").ap()
    xc_t = nc.dram_tensor("xc_t", [E, T], BF, kind="ExternalInput").ap()
    w_vin = nc.dram_tensor("w_vin", [E, 3 * E], BF, kind="ExternalInput").ap()
    w_hq = nc.dram_tensor("w_hq", [E, E], BF, kind="ExternalInput").ap()
    w_hkv = nc.dram_tensor("w_hkv", [E, 2 * E], BF, kind="ExternalInput").ap()
    w_fh = nc.dram_tensor("w_fh", [E, E], BF, kind="ExternalInput").ap()
    w_fv = nc.dram_tensor("w_fv", [E, E], BF, kind="ExternalInput").ap()
    w_m2 = nc.dram_tensor("w_m2", [E, E], BF, kind="ExternalInput").ap()
    mask_d = nc.dram_tensor("mask", [128, 448], BF, kind="ExternalInput").ap()
    bias_d = nc.dram_tensor("biases", [128, 32], F32, kind="ExternalInput").ap()
    out_t = nc.dram_tensor("out_t", [E, T], F32, kind="ExternalOutput").ap()

    with tile.TileContext(nc) as tc, ExitStack() as ctx:
        pw = ctx.enter_context(tc.tile_pool(name="pw", bufs=1))
        psA = ctx.enter_context(tc.tile_pool(name="psA", bufs=2, space="PSUM"))
        psS = ctx.enter_context(tc.tile_pool(name="psS", bufs=2, space="PSUM"))
        psC = ctx.enter_context(tc.tile_pool(name="psC", bufs=2, space="PSUM"))
        px = ctx.enter_context(tc.tile_pool(name="px", bufs=3))
        pqk = ctx.enter_context(tc.tile_pool(name="pqk", bufs=32))
        pv = ctx.enter_context(tc.tile_pool(name="pv", bufs=8))
        pp = ctx.enter_context(tc.tile_pool(name="pp", bufs=12))
        pctx = ctx.enter_context(tc.tile_pool(name="pctx", bufs=8))
        pzr = ctx.enter_context(tc.tile_pool(name="pzr", bufs=8))
        pct = ctx.enter_context(tc.tile_pool(name="pct", bufs=16))
        phid = ctx.enter_context(tc.tile_pool(name="phid", bufs=8))
        pout = ctx.enter_context(tc.tile_pool(name="pout", bufs=8))

        # ---- persistent constants ----
        def load_const(name, dram_ap, shape, dtype):
            t = pw.tile(shape, dtype, tag=name)
            nc.gpsimd.dma_start(t[:], dram_ap)
            return t

        wv = [load_const(f"wv{k}", w_vin[128 * k:128 * (k + 1), :], [128, 3 * E], BF)
              for k in range(4)]
        whq = [load_const(f"whq{k}", w_hq[128 * k:128 * (k + 1), :], [128, E], BF)
               for k in range(4)]
        whkv = [load_const(f"whkv{k}", w_hkv[128 * k:128 * (k + 1), :], [128, 2 * E], BF)
                for k in range(4)]
        wfh = [load_const(f"wfh{k}", w_fh[128 * k:128 * (k + 1), :], [128, E], BF)
               for k in range(4)]
        wfv = [load_const(f"wfv{k}", w_fv[128 * k:128 * (k + 1), :], [128, E], BF)
               for k in range(4)]
        wm2 = [load_const(f"wm2{k}", w_m2[128 * k:128 * (k + 1), :], [128, E], BF)
               for k in range(4)]
        msk = load_const("msk", mask_d[:, :], [128, 448], BF)
        bia = load_const("bia", bias_d[:, :], [128, 32], F32)

        # bias column map (within `bia`):
        # 0-7 v_in_b[0:1024] ftiles; 8-11 h_in_b[0:512]; 12-15 h_in_b[512:1024]
        # 24-27 c1 (fused W1@out_bias + mlp_b1); 28-31 mlp_b2

        # per-pair state, filled by the emit helpers
        xr2_all = [None] * NPAIR
        xc2_all = [None] * NPAIR
        qk_all = [None] * NPAIR
        ct_all = [None] * NPAIR

        def emit_dma(g):
            g0 = 2 * S * g
            xr2, xc2 = [], []
            for k in range(4):
                t = px.tile([128, 2 * S], BF, tag=f"xr{k}")
                nc.gpsimd.dma_start(t[:], xr_t[128 * k:128 * (k + 1), g0:g0 + 2 * S])
                xr2.append(t)
                t = px.tile([128, 2 * S], BF, tag=f"xc{k}")
                nc.gpsimd.dma_start(t[:], xc_t[128 * k:128 * (k + 1), g0:g0 + 2 * S])
                xc2.append(t)
            xr2_all[g] = xr2
            xc2_all[g] = xc2

        def emit_qk(g, branch):
            """QK projections for one branch of pair g, feature-major, N=384.
            ftile j in 0..7: j<4 -> Q features, j>=4 -> K features."""
            if qk_all[g] is None:
                qk_all[g] = {}
            xr2, xc2 = xr2_all[g], xc2_all[g]
            qk_all[g][branch] = []
            for j in range(8):
                ps = psA.tile([128, 384], F32, tag="proj",
                              padded_shape=[128, 512])
                for k in range(4):
                    if branch == "v":
                        lhsT = wv[k][:, 128 * j:128 * (j + 1)]
                        rhs = xr2[k][:]
                    elif j < 4:   # h Q
                        lhsT = whq[k][:, 128 * j:128 * (j + 1)]
                        rhs = xr2[k][:]
                    else:         # h K
                        lhsT = whkv[k][:, 128 * (j - 4):128 * (j - 3)]
                        rhs = xc2[k][:]
                    nc.tensor.matmul(ps[:], lhsT=lhsT, rhs=rhs,
                                     start=(k == 0), stop=(k == 3))
                # bias columns: v ftiles 0-7 -> cols 0-7; h Q 0-3 -> 8-11;
                # h K 0-3 -> 12-15
                dst = pqk.tile([128, 384], BF, tag="qk")
                need_b = (has_vqk_b if branch == "v"
                          else (has_hq_b if j < 4 else has_hk_b))
                if need_b:
                    bcol = j if branch == "v" else (8 + j)
                    nc.scalar.activation(dst[:], ps[:], AF.Identity,
                                         bias=bia[:, bcol:bcol + 1])
                else:
                    nc.scalar.activation(dst[:], ps[:], AF.Copy)
                qk_all[g][branch].append(dst)

        def emit_attn_sb(g, a, br):
            """Attention for strip a (0/1) of pair g, branch br."""
            s0 = S * a
            xin = xr2_all[g] if br == "v" else xc2_all[g]
            vcols = slice(1024, 1536) if br == "v" else slice(512, 1024)
            vw = wv if br == "v" else whkv
            qk = qk_all[g][br]
            ct = ct_all[g]

            # ------ V projection, token-major, with ones column ----
            vps_a = psA.tile([128, 512], F32, tag="proj")
            vps_b = psA.tile([128, 512], F32, tag="proj")
            for k in range(4):
                nc.tensor.matmul(vps_a[:], lhsT=xin[k][:, s0:s0 + 128],
                                 rhs=vw[k][:, vcols],
                                 start=(k == 0), stop=(k == 3))
            for k in range(4):
                nc.tensor.matmul(vps_b[:], lhsT=xin[k][:, s0 + 64:s0 + 192],
                                 rhs=vw[k][:, vcols],
                                 start=(k == 0), stop=(k == 3))
            va = pv.tile([128, 8, 65], BF, tag="vp")   # keys [0:128)
            vb = pv.tile([128, 8, 65], BF, tag="vp")   # keys [64:192)
            nc.vector.tensor_copy(
                va[:, :, 0:64],
                vps_a[:].rearrange("p (h c) -> p h c", c=64))
            nc.vector.tensor_copy(
                vb[:, :, 0:64],
                vps_b[:].rearrange("p (h c) -> p h c", c=64))
            nc.vector.memset(va[:, :, 64:65], 1.0)
            nc.vector.memset(vb[:, :, 64:65], 1.0)

            # ------ scores + exp + mask for all 4 head pairs ------
            pms = []
            for p in range(4):
                QT = qk[p][:, s0:s0 + S]
                KT = qk[4 + p][:, s0:s0 + S]
                # one 2-bank tile per head pair; head h2 -> bank h2.
                # Matmuls with disjoint contraction row-groups (head0 at
                # partitions 0:64, head1 at 64:128) run CONCURRENTLY on the
                # PE and hard-fault if they write the same PSUM bank.
                sp = psS.tile([128, 2, 512], F32, tag="sc")
                for h2 in range(2):
                    d0 = 64 * h2
                    nc.tensor.matmul(sp[:, h2:h2 + 1, 0:128],
                                     lhsT=KT[d0:d0 + 64, 0:128],
                                     rhs=QT[d0:d0 + 64, 0:128],
                                     start=True, stop=True)
                    nc.tensor.matmul(sp[:, h2:h2 + 1, 128:224],
                                     lhsT=KT[d0:d0 + 64, 64:192],
                                     rhs=QT[d0:d0 + 64, 96:192],
                                     start=True, stop=True)
                # ONE exp for both heads: strided [128, 2, 224] PSUM read
                pb = pp.tile([128, 512], BF, tag="p")
                nc.scalar.activation(
                    pb[:, 0:448].rearrange("p (b c) -> p b c", b=2),
                    sp[:, :, 0:224], AF.Exp, scale=0.125)
                pm = pp.tile([128, 512], BF, tag="p")
                nc.vector.tensor_tensor(pm[:, 0:448], pb[:, 0:448],
                                        msk[:, 0:448], op=MUL)
                pms.append(pm)

            # ------ attn@V + normalize + ctx transpose per head pair ------
            for p in range(4):
                pm = pms[p]
                # attn@V, one accumulation "group" per bank, ordered by
                # _chain. All matmuls use base-0 contraction rows padded to
                # overlapping ranges (the pad rows are band-mask zeros in pm),
                # so none of them can run concurrently and collide on the
                # bank. The built-in group checker cannot express
                # multi-region banks, so skip it; correctness comes from the
                # explicit ordering + per-element pending-zero semantics.
                cp = psC.tile([128, 512], F32, tag="cx")
                mms = []
                for h2 in range(2):
                    h = 2 * p + h2
                    cb = 130 * h2
                    ta = 224 * h2
                    tb = 224 * h2 + 128
                    # q in [0,128): keys [0:128) from TA
                    mms.append(nc.tensor.matmul(
                        cp[:, cb:cb + 65], lhsT=pm[:, ta:ta + 128],
                        rhs=va[:, h:h + 1, :], start=(h2 == 0),
                        stop=False, skip_group_check=True))
                    # q in [96,128): keys [96:160) = TB rows [32:96),
                    # K padded to rows [0:96) (rows 0:32 masked 0)
                    mms.append(nc.tensor.matmul(
                        cp[96:128, cb:cb + 65],
                        lhsT=pm[0:96, tb:tb + 32],
                        rhs=vb[0:96, h:h + 1, :],
                        start=False, stop=False, tile_position=(0, 96),
                        skip_group_check=True))
                    # q in [128,192): keys [96:192) = TB rows [32:128),
                    # K padded to rows [0:128)
                    mms.append(nc.tensor.matmul(
                        cp[0:64, cb + 65:cb + 130],
                        lhsT=pm[0:128, tb + 32:tb + 96],
                        rhs=vb[0:128, h:h + 1, :],
                        start=False, stop=(h2 == 1),
                        skip_group_check=True))
                _chain(mms)

                # normalize by 1/Z (Z = ones-column accumulation at cols
                # {64, 129, 194, 259} = 64 + 130h + 65q) and pack for the
                # transpose: ctxn = [h0q1 | h1q1 | h0q2 | h1q2], 64 cols each
                zr = pzr.tile([128, 2, 2, 1], F32, tag="zr")
                zin = (cp[:, 64:324]
                       .rearrange("p (h x) -> p h x", h=2)
                       .rearrange("p h (q c) -> p h q c", q=2))
                ctxn = pctx.tile([128, 256], BF, tag="ctxn")
                reads = [
                    nc.vector.reciprocal(zr[:], zin[:, :, :, 0:1]),
                ]
                # 4 scale ops: 2 on DVE, 2 on ACT for engine balance
                reads.append(nc.vector.tensor_scalar_mul(
                    ctxn[:, 0:64], cp[:, 0:64], zr[:, 0:1, 0:1, :]))
                reads.append(nc.scalar.activation(
                    ctxn[:, 64:128], cp[:, 130:194], AF.Identity,
                    scale=zr[:, 1:2, 0:1, :]))
                reads.append(nc.vector.tensor_scalar_mul(
                    ctxn[0:64, 128:192], cp[0:64, 65:129],
                    zr[0:64, 0:1, 1:2, :]))
                reads.append(nc.scalar.activation(
                    ctxn[0:64, 192:256], cp[0:64, 195:259], AF.Identity,
                    scale=zr[0:64, 1:2, 1:2, :]))
                # cp reads must wait for the accumulation group to close
                # (same-bank PE-write + DVE-read is a HW fault)
                for r in reads:
                    add_dep_helper(r.ins, mms[-1].ins, sync=True,
                                   reason="psum read after group close")

                # ctx^T via the DMA xbar transpose engine (SBUF->SBUF),
                # keeping the PE free for matmuls
                ct_p = ct[(0 if br == "h" else 4) + p]
                nc.sync.dma_start_transpose(ct_p[:, s0:s0 + 128],
                                            ctxn[:, 0:128])
                nc.sync.dma_start_transpose(ct_p[:, s0 + 128:s0 + 192],
                                            ctxn[0:64, 128:256])

        def emit_mlp(g):
            """Fused (out-proj + MLP1) then MLP2 for pair g, N=384."""
            ct = ct_all[g]
            g0 = 2 * S * g
            hid = []
            for j in range(4):
                ps = psA.tile([128, 384], F32, tag="proj",
                              padded_shape=[128, 512])
                mms = []
                for k in range(4):
                    mms.append(nc.tensor.matmul(
                        ps[:], lhsT=wfh[k][:, 128 * j:128 * (j + 1)],
                        rhs=ct[k][:], start=(k == 0), stop=False))
                for k in range(4):
                    mms.append(nc.tensor.matmul(
                        ps[:], lhsT=wfv[k][:, 128 * j:128 * (j + 1)],
                        rhs=ct[4 + k][:], start=False, stop=(k == 3)))
                dst = phid.tile([128, 384], BF, tag="hid")
                if has_c1:
                    nc.scalar.activation(dst[:], ps[:], AF.Relu,
                                         bias=bia[:, 24 + j:24 + j + 1])
                else:
                    nc.scalar.activation(dst[:], ps[:], AF.Relu)
                hid.append(dst)
            for j in range(4):
                ps = psA.tile([128, 384], F32, tag="proj",
                              padded_shape=[128, 512])
                for k in range(4):
                    nc.tensor.matmul(ps[:],
                                     lhsT=wm2[k][:, 128 * j:128 * (j + 1)],
                                     rhs=hid[k][:],
                                     start=(k == 0), stop=(k == 3))
                osb = pout.tile([128, 384], F32, tag="o")
                if has_b2:
                    nc.scalar.activation(osb[:], ps[:], AF.Identity,
                                         bias=bia[:, 28 + j:28 + j + 1])
                else:
                    nc.scalar.activation(osb[:], ps[:], AF.Copy)
                nc.sync.dma_start(out_t[128 * j:128 * (j + 1), g0:g0 + 2 * S],
                                  osb[:])

        # ---- software-pipelined emission ----
        # attention of pair g interleaves with QK projections of pair g+1
        # (dense N=384 matmuls) so the PE array duty stays high and HAM
        # keeps the 2.4 GHz clock.
        emit_dma(0)
        emit_dma(1)
        for g in range(NPAIR):
            ct_all[g] = [pct.tile([128, 2 * S], BF, tag="ct",
                                  name=f"ct_{g}_{i}") for i in range(8)]
            if g == 0:
                emit_qk(0, "h")
                emit_qk(0, "v")
            if g + 2 < NPAIR:
                emit_dma(g + 2)
            for a in range(2):
                emit_attn_sb(g, a, "h")
                emit_attn_sb(g, a, "v")
                if g + 1 < NPAIR:
                    emit_qk(g + 1, "h" if a == 0 else "v")
            emit_mlp(g)
    nc.finalize()
    return nc


_CACHE = {}


def _get_program(bias_flags):
    key = tuple(bias_flags)
    if key not in _CACHE:
        _CACHE[key] = _build_program(key)
    return _CACHE[key]


def _col(b):
    """bias vector (128*n,) -> (128, n) column-pack, fortran-ish layout."""
    return np.ascontiguousarray(b.reshape(-1, 128).T.astype(np.float32))


def kernel(hidden_states, h_in_w, h_in_b, h_out_w, h_out_b,
           v_in_w, v_in_b, v_out_w, v_out_b,
           mlp_w1, mlp_b1, mlp_w2, mlp_b2):
    x = np.asarray(hidden_states, dtype=np.float32)
    h_in_w = np.asarray(h_in_w, np.float32)
    h_in_b = np.asarray(h_in_b, np.float32)
    h_out_w = np.asarray(h_out_w, np.float32)
    h_out_b = np.asarray(h_out_b, np.float32)
    v_in_w = np.asarray(v_in_w, np.float32)
    v_in_b = np.asarray(v_in_b, np.float32)
    v_out_w = np.asarray(v_out_w, np.float32)
    v_out_b = np.asarray(v_out_b, np.float32)
    mlp_w1 = np.asarray(mlp_w1, np.float32)
    mlp_b1 = np.asarray(mlp_b1, np.float32)
    mlp_w2 = np.asarray(mlp_w2, np.float32)
    mlp_b2 = np.asarray(mlp_b2, np.float32)

    # V biases act as a constant shift of ctx (softmax weights sum to 1),
    # so fold them through the out-projections; then fold the
    # out-projections themselves into MLP1 (everything is linear up to the
    # ReLU): hid = relu(W1h(Who ctx_h + bho) + W1v(Wvo ctx_v + bvo) + b1)
    #            = relu(F_h ctx_h + F_v ctx_v + c1)
    h_out_eff = h_out_b + h_out_w @ h_in_b[2 * E:3 * E]
    v_out_eff = v_out_b + v_out_w @ v_in_b[2 * E:3 * E]
    w1h = mlp_w1[:, 0:E]
    w1v = mlp_w1[:, E:2 * E]
    f_h = w1h @ h_out_w          # (E, E): hid += F_h @ ctx_h
    f_v = w1v @ v_out_w
    c1 = w1h @ h_out_eff + w1v @ v_out_eff + mlp_b1

    bias_flags = (
        bool(np.any(v_in_b[0:2 * E])), bool(np.any(h_in_b[0:E])),
        bool(np.any(h_in_b[E:2 * E])), bool(np.any(c1)), bool(np.any(mlp_b2)),
    )
    nc = _get_program(bias_flags)

    biases = np.zeros((128, 32), np.float32)
    biases[:, 0:8] = _col(v_in_b[0:2 * E])
    biases[:, 8:16] = _col(h_in_b[0:2 * E])
    biases[:, 24:28] = _col(c1)
    biases[:, 28:32] = _col(mlp_b2)

    shared = {
        "w_vin": np.ascontiguousarray(v_in_w.T).astype(NPBF),
        "w_hq": np.ascontiguousarray(h_in_w[0:E].T).astype(NPBF),
        "w_hkv": np.ascontiguousarray(h_in_w[E:3 * E].T).astype(NPBF),
        "w_fh": np.ascontiguousarray(f_h.T).astype(NPBF),
        "w_fv": np.ascontiguousarray(f_v.T).astype(NPBF),
        "w_m2": np.ascontiguousarray(mlp_w2.T).astype(NPBF),
        "mask": _band_masks(),
        "biases": biases,
    }

    in_maps = []
    for c in range(NCORE):
        rows = x[RPC * c:RPC * (c + 1)]                      # (24, 192, 512)
        cols = x[:, RPC * c:RPC * (c + 1)].transpose(1, 0, 2)  # (24, 192, 512)
        m = dict(shared)
        m["xr_t"] = np.ascontiguousarray(rows.reshape(T, E).T).astype(NPBF)
        m["xc_t"] = np.ascontiguousarray(cols.reshape(T, E).T).astype(NPBF)
        in_maps.append(m)

    global _LAST_IN_MAPS
    _LAST_IN_MAPS = in_maps
    res = run_bass_kernel_spmd(nc, in_maps, core_ids=list(range(NCORE)))

    out = np.empty((S, S, E), np.float32)
    for c in range(NCORE):
        out[RPC * c:RPC * (c + 1)] = res.results[c]["out_t"].T.reshape(RPC, S, E)
    return out


# revision 10
# speedup vs baseline: 1.2657x; 1.2657x over previous
"""BiSPA (bidirectional sparse windowed attention + MLP) Trainium2 kernel.

Full inputs in, full outputs out. Internally shards across 8 NeuronCores:
core c owns output rows Ic = [24c, 24c+24) of the (192, 192, 512) grid.

Key observation: with B == S == 192 and window W == 32,
  - vertical attention for output row i is a complete 192-token sliding-window
    attention over x[i, :, :]                        (needs x rows  Ic)
  - horizontal attention for output row i is a complete 192-token
    sliding-window attention with Q from x[i, :, :] and K/V from x[:, i, :]
    (needs x columns Ic)
so each core needs x[Ic, :, :] and x[:, Ic, :] and NOTHING else -> zero
duplicated projection FLOPs, zero collectives, no halos.

v2 changes vs the first working kernel (tensor-engine bound at ~864us):
  - out-projections folded into MLP1 on the host: F_h = W1h @ Who,
    F_v = W1v @ Wvo, c1 = W1h@h_out_eff + W1v@v_out_eff + b1. Removes
    384 N=384 matmuls + 8 evictions per pair.
  - scores for a head pair land in one 2-bank PSUM tile -> ONE exp
    activation per head pair (batched [128, 2, 224] strided read).
  - softmax normalization: one batched reciprocal per head pair
    ([128,2,2,1] AP over the 4 Z columns), 4 DVE scale ops.
  - ctx^T transposes stay on the PE but write f32 into SPARE COLUMNS
    (260:452) of the same PSUM bank as the attn@V accumulator, so no
    extra PSUM banks are needed; ONE ACT copy per head pair evicts.
  - emission software-pipelined: QK projections of pair g+1 interleave
    with the attention inner loop of pair g so the PE array duty never
    dips long enough for HAM to re-throttle the clock (354us of the
    baseline ran at 1.2 GHz instead of 2.4); MLP of pair g is emitted
    during pair g+1 so it never waits on fresh ctx evictions.

Numerics: all matmul inputs bf16, fp32 PSUM accumulation, fp32 softmax exp
input, bf16 probs/ctx.  Measured end-to-end rel err vs fp32 reference ~0.6%.
"""

import numpy as np
from contextlib import ExitStack

import concourse.bass as bass
import concourse.mybir as mybir
import concourse.tile as tile
from concourse import bacc
from concourse.bass_utils import run_bass_kernel_spmd
from concourse.masks import make_identity
from concourse.tile import add_dep_helper


def _chain(insts):
    """Order matmuls targeting one PSUM bank: a start=True zeroes (marks
    pending-zero) the WHOLE 2KB bank, so each bank must hold exactly one
    accumulation group and the group's matmuls must execute in program order.
    Tile won't order disjoint-region writes by itself."""
    for a, b in zip(insts, insts[1:]):
        add_dep_helper(b.ins, a.ins, sync=False, reason="psum-bank group order")

BF = mybir.dt.bfloat16
F32 = mybir.dt.float32
AF = mybir.ActivationFunctionType
MUL = mybir.AluOpType.mult
NPBF = mybir.dt.np(BF)

E = 512
H = 8
D = 64
W = 32
S = 192
NCORE = 8
RPC = 24          # rows (strips) per core
T = RPC * S       # tokens per core per branch = 4608
NPAIR = RPC // 2  # strip pairs per core


def _band_masks():
    """Score mask, bf16 (128, 448): [TA 128 | TB 96 | pad] x 2 heads.

    TA: rows p = key k in [0,128), cols q in [0,128).
        valid = |k-q| <= W and not (k >= 96 and q >= 96)
    TB: rows p -> key k = 64+p in [64,192), cols q' -> q = 96+q' in [96,192).
        valid = |k-q| <= W and k >= 96
    """
    m = np.zeros((128, 224), np.float32)
    k = np.arange(128)[:, None]
    q = np.arange(128)[None, :]
    ta = (np.abs(k - q) <= W) & ~((k >= 96) & (q >= 96))
    kb = 64 + np.arange(128)[:, None]
    qb = 96 + np.arange(96)[None, :]
    tb = (np.abs(kb - qb) <= W) & (kb >= 96)
    m[:, 0:128] = ta
    m[:, 128:224] = tb
    return np.concatenate([m, m], axis=1).astype(NPBF)


def _build_program(bias_flags):
    """Build the SPMD Bass/Tile program (same program on all 8 cores)."""
    has_vqk_b, has_hq_b, has_hk_b, has_c1, has_b2 = bias_flags

    nc = bacc.Bacc("TRN2", target_bir_lowering=False, debug=False,
                   num_devices=NCORE, num_swdge_queues=4)

    xr_t = nc.dram_tensor("xr_t", [E, T], BF, kind="ExternalInput").ap()
    xc_t = nc.dram_tensor("xc_t", [E, T], BF, kind="ExternalInput").ap()
    w_vin = nc.dram_tensor("w_vin", [E, 3 * E], BF, kind="ExternalInput").ap()
    w_hq = nc.dram_tensor("w_hq", [E, E], BF, kind="ExternalInput").ap()
    w_hkv = nc.dram_tensor("w_hkv", [E, 2 * E], BF, kind="ExternalInput").ap()
    w_fh = nc.dram_tensor("w_fh", [E, E], BF, kind="ExternalInput").ap()
    w_fv = nc.dram_tensor("w_fv", [E, E], BF, kind="ExternalInput").ap()
    w_m2 = nc.dram_tensor("w_m2", [E, E], BF, kind="ExternalInput").ap()
    mask_d = nc.dram_tensor("mask", [128, 448], BF, kind="ExternalInput").ap()
    bias_d = nc.dram_tensor("biases", [128, 32], F32, kind="ExternalInput").ap()
    out_t = nc.dram_tensor("out_t", [E, T], F32, kind="ExternalOutput").ap()

    with tile.TileContext(nc) as tc, ExitStack() as ctx:
        pw = ctx.enter_context(tc.tile_pool(name="pw", bufs=1))
        psA = ctx.enter_context(tc.tile_pool(name="psA", bufs=2, space="PSUM"))
        psS = ctx.enter_context(tc.tile_pool(name="psS", bufs=2, space="PSUM"))
        psC = ctx.enter_context(tc.tile_pool(name="psC", bufs=2, space="PSUM"))
        px = ctx.enter_context(tc.tile_pool(name="px", bufs=3))
        pqk = ctx.enter_context(tc.tile_pool(name="pqk", bufs=32))
        pv = ctx.enter_context(tc.tile_pool(name="pv", bufs=8))
        pp = ctx.enter_context(tc.tile_pool(name="pp", bufs=12))
        pctx = ctx.enter_context(tc.tile_pool(name="pctx", bufs=8))
        pzr = ctx.enter_context(tc.tile_pool(name="pzr", bufs=8))
        pct = ctx.enter_context(tc.tile_pool(name="pct", bufs=16))
        phid = ctx.enter_context(tc.tile_pool(name="phid", bufs=8))
        pout = ctx.enter_context(tc.tile_pool(name="pout", bufs=8))

        # ---- persistent constants ----
        def load_const(name, dram_ap, shape, dtype):
            t = pw.tile(shape, dtype, tag=name)
            nc.gpsimd.dma_start(t[:], dram_ap)
            return t

        wv = [load_const(f"wv{k}", w_vin[128 * k:128 * (k + 1), :], [128, 3 * E], BF)
              for k in range(4)]
        whq = [load_const(f"whq{k}", w_hq[128 * k:128 * (k + 1), :], [128, E], BF)
               for k in range(4)]
        whkv = [load_const(f"whkv{k}", w_hkv[128 * k:128 * (k + 1), :], [128, 2 * E], BF)
                for k in range(4)]
        wfh = [load_const(f"wfh{k}", w_fh[128 * k:128 * (k + 1), :], [128, E], BF)
               for k in range(4)]
        wfv = [load_const(f"wfv{k}", w_fv[128 * k:128 * (k + 1), :], [128, E], BF)
               for k in range(4)]
        wm2 = [load_const(f"wm2{k}", w_m2[128 * k:128 * (k + 1), :], [128, E], BF)
               for k in range(4)]
        msk = load_const("msk", mask_d[:, :], [128, 448], BF)
        bia = load_const("bia", bias_d[:, :], [128, 32], F32)
        ident = pw.tile([128, 128], BF, tag="ident")
        make_identity(nc, ident)

        # bias column map (within `bia`):
        # 0-7 v_in_b[0:1024] ftiles; 8-11 h_in_b[0:512]; 12-15 h_in_b[512:1024]
        # 24-27 c1 (fused W1@out_bias + mlp_b1); 28-31 mlp_b2

        # per-pair state, filled by the emit helpers
        xr2_all = [None] * NPAIR
        xc2_all = [None] * NPAIR
        qk_all = [None] * NPAIR
        ct_all = [None] * NPAIR

        def emit_dma(g):
            g0 = 2 * S * g
            xr2, xc2 = [], []
            for k in range(4):
                t = px.tile([128, 2 * S], BF, tag=f"xr{k}")
                nc.gpsimd.dma_start(t[:], xr_t[128 * k:128 * (k + 1), g0:g0 + 2 * S])
                xr2.append(t)
                t = px.tile([128, 2 * S], BF, tag=f"xc{k}")
                nc.gpsimd.dma_start(t[:], xc_t[128 * k:128 * (k + 1), g0:g0 + 2 * S])
                xc2.append(t)
            xr2_all[g] = xr2
            xc2_all[g] = xc2

        def emit_qk(g, branch):
            """QK projections for one branch of pair g, feature-major, N=384.
            ftile j in 0..7: j<4 -> Q features, j>=4 -> K features."""
            if qk_all[g] is None:
                qk_all[g] = {}
            xr2, xc2 = xr2_all[g], xc2_all[g]
            qk_all[g][branch] = []
            for j in range(8):
                ps = psA.tile([128, 384], F32, tag="proj",
                              padded_shape=[128, 512])
                for k in range(4):
                    if branch == "v":
                        lhsT = wv[k][:, 128 * j:128 * (j + 1)]
                        rhs = xr2[k][:]
                    elif j < 4:   # h Q
                        lhsT = whq[k][:, 128 * j:128 * (j + 1)]
                        rhs = xr2[k][:]
                    else:         # h K
                        lhsT = whkv[k][:, 128 * (j - 4):128 * (j - 3)]
                        rhs = xc2[k][:]
                    nc.tensor.matmul(ps[:], lhsT=lhsT, rhs=rhs,
                                     start=(k == 0), stop=(k == 3))
                # bias columns: v ftiles 0-7 -> cols 0-7; h Q 0-3 -> 8-11;
                # h K 0-3 -> 12-15
                dst = pqk.tile([128, 384], BF, tag="qk")
                need_b = (has_vqk_b if branch == "v"
                          else (has_hq_b if j < 4 else has_hk_b))
                if need_b:
                    bcol = j if branch == "v" else (8 + j)
                    nc.scalar.activation(dst[:], ps[:], AF.Identity,
                                         bias=bia[:, bcol:bcol + 1])
                else:
                    nc.scalar.activation(dst[:], ps[:], AF.Copy)
                qk_all[g][branch].append(dst)

        def emit_attn_sb(g, a, br):
            """Attention for strip a (0/1) of pair g, branch br."""
            s0 = S * a
            xin = xr2_all[g] if br == "v" else xc2_all[g]
            vcols = slice(1024, 1536) if br == "v" else slice(512, 1024)
            vw = wv if br == "v" else whkv
            qk = qk_all[g][br]
            ct = ct_all[g]

            # ------ V projection, token-major, with ones column ----
            vps_a = psA.tile([128, 512], F32, tag="proj")
            vps_b = psA.tile([128, 512], F32, tag="proj")
            for k in range(4):
                nc.tensor.matmul(vps_a[:], lhsT=xin[k][:, s0:s0 + 128],
                                 rhs=vw[k][:, vcols],
                                 start=(k == 0), stop=(k == 3))
            for k in range(4):
                nc.tensor.matmul(vps_b[:], lhsT=xin[k][:, s0 + 64:s0 + 192],
                                 rhs=vw[k][:, vcols],
                                 start=(k == 0), stop=(k == 3))
            va = pv.tile([128, 8, 65], BF, tag="vp")   # keys [0:128)
            vb = pv.tile([128, 8, 65], BF, tag="vp")   # keys [64:192)
            nc.vector.tensor_copy(
                va[:, :, 0:64],
                vps_a[:].rearrange("p (h c) -> p h c", c=64))
            nc.vector.tensor_copy(
                vb[:, :, 0:64],
                vps_b[:].rearrange("p (h c) -> p h c", c=64))
            nc.vector.memset(va[:, :, 64:65], 1.0)
            nc.vector.memset(vb[:, :, 64:65], 1.0)

            # ------ scores + exp + mask for all 4 head pairs ------
            pms = []
            for p in range(4):
                QT = qk[p][:, s0:s0 + S]
                KT = qk[4 + p][:, s0:s0 + S]
                # one 2-bank tile per head pair; head h2 -> bank h2.
                # Matmuls with disjoint contraction row-groups (head0 at
                # partitions 0:64, head1 at 64:128) run CONCURRENTLY on the
                # PE and hard-fault if they write the same PSUM bank.
                sp = psS.tile([128, 2, 512], F32, tag="sc")
                for h2 in range(2):
                    d0 = 64 * h2
                    nc.tensor.matmul(sp[:, h2:h2 + 1, 0:128],
                                     lhsT=KT[d0:d0 + 64, 0:128],
                                     rhs=QT[d0:d0 + 64, 0:128],
                                     start=True, stop=True)
                    nc.tensor.matmul(sp[:, h2:h2 + 1, 128:224],
                                     lhsT=KT[d0:d0 + 64, 64:192],
                                     rhs=QT[d0:d0 + 64, 96:192],
                                     start=True, stop=True)
                # ONE exp for both heads: strided [128, 2, 224] PSUM read
                pb = pp.tile([128, 512], BF, tag="p")
                nc.scalar.activation(
                    pb[:, 0:448].rearrange("p (b c) -> p b c", b=2),
                    sp[:, :, 0:224], AF.Exp, scale=0.125)
                pm = pp.tile([128, 512], BF, tag="p")
                nc.vector.tensor_tensor(pm[:, 0:448], pb[:, 0:448],
                                        msk[:, 0:448], op=MUL)
                pms.append(pm)

            # ------ attn@V + normalize + ctx transpose per head pair ------
            for p in range(4):
                pm = pms[p]
                # attn@V, one accumulation "group" per bank, ordered by
                # _chain. All matmuls use base-0 contraction rows padded to
                # overlapping ranges (the pad rows are band-mask zeros in pm),
                # so none of them can run concurrently and collide on the
                # bank. The built-in group checker cannot express
                # multi-region banks, so skip it; correctness comes from the
                # explicit ordering + per-element pending-zero semantics.
                cp = psC.tile([128, 512], F32, tag="cx")
                mms = []
                for h2 in range(2):
                    h = 2 * p + h2
                    cb = 130 * h2
                    ta = 224 * h2
                    tb = 224 * h2 + 128
                    # q in [0,128): keys [0:128) from TA
                    mms.append(nc.tensor.matmul(
                        cp[:, cb:cb + 65], lhsT=pm[:, ta:ta + 128],
                        rhs=va[:, h:h + 1, :], start=(h2 == 0),
                        stop=False, skip_group_check=True))
                    # q in [96,128): keys [96:160) = TB rows [32:96),
                    # K padded to rows [0:96) (rows 0:32 masked 0)
                    mms.append(nc.tensor.matmul(
                        cp[96:128, cb:cb + 65],
                        lhsT=pm[0:96, tb:tb + 32],
                        rhs=vb[0:96, h:h + 1, :],
                        start=False, stop=False, tile_position=(0, 96),
                        skip_group_check=True))
                    # q in [128,192): keys [96:192) = TB rows [32:128),
                    # K padded to rows [0:128)
                    mms.append(nc.tensor.matmul(
                        cp[0:64, cb + 65:cb + 130],
                        lhsT=pm[0:128, tb + 32:tb + 96],
                        rhs=vb[0:128, h:h + 1, :],
                        start=False, stop=(h2 == 1),
                        skip_group_check=True))
                _chain(mms)

                # normalize by 1/Z (Z = ones-column accumulation at cols
                # {64, 129, 194, 259} = 64 + 130h + 65q) and pack for the
                # transpose: ctxn = [h0q1 | h1q1 | h0q2 | h1q2], 64 cols each
                zr = pzr.tile([128, 2, 2, 1], F32, tag="zr")
                zin = (cp[:, 64:324]
                       .rearrange("p (h x) -> p h x", h=2)
                       .rearrange("p h (q c) -> p h q c", q=2))
                ctxn = pctx.tile([128, 256], BF, tag="ctxn")
                reads = [
                    nc.vector.reciprocal(zr[:], zin[:, :, :, 0:1]),
                    nc.vector.tensor_scalar_mul(
                        ctxn[:, 0:64], cp[:, 0:64], zr[:, 0:1, 0:1, :]),
                    nc.vector.tensor_scalar_mul(
                        ctxn[:, 64:128], cp[:, 130:194], zr[:, 1:2, 0:1, :]),
                    nc.vector.tensor_scalar_mul(
                        ctxn[0:64, 128:192], cp[0:64, 65:129],
                        zr[0:64, 0:1, 1:2, :]),
                    nc.vector.tensor_scalar_mul(
                        ctxn[0:64, 192:256], cp[0:64, 195:259],
                        zr[0:64, 1:2, 1:2, :]),
                ]
                # cp reads must wait for the accumulation group to close
                # (same-bank PE-write + DVE-read is a HW fault)
                for r in reads:
                    add_dep_helper(r.ins, mms[-1].ins, sync=True,
                                   reason="psum read after group close")

                # ctx^T via PE transpose, writing f32 into the SPARE columns
                # (260:452) of the same PSUM bank as cp -- no extra banks.
                # The transposes must not overlap the DVE normalize reads of
                # this bank (PE-write + DVE-read same bank is a HW fault).
                tps = [
                    nc.tensor.transpose(cp[:, 260:324].bitcast(BF),
                                        ctxn[:, 0:128], ident[:]),
                    nc.tensor.transpose(cp[:, 324:356].bitcast(BF),
                                        ctxn[0:64, 128:256],
                                        ident[0:64, 0:64]),
                ]
                for t in tps:
                    for r in reads:
                        add_dep_helper(t.ins, r.ins, sync=True,
                                       reason="transpose after bank reads")
                ct_p = ct[(0 if br == "h" else 4) + p]
                ev = nc.scalar.activation(ct_p[:, s0:s0 + S],
                                          cp[:, 260:356].bitcast(BF), AF.Copy)
                for t in tps:
                    add_dep_helper(ev.ins, t.ins, sync=True,
                                   reason="evict after transpose")

        def emit_mlp(g):
            """Fused (out-proj + MLP1) then MLP2 for pair g, N=384."""
            ct = ct_all[g]
            g0 = 2 * S * g
            hid = []
            for j in range(4):
                ps = psA.tile([128, 384], F32, tag="proj",
                              padded_shape=[128, 512])
                mms = []
                for k in range(4):
                    mms.append(nc.tensor.matmul(
                        ps[:], lhsT=wfh[k][:, 128 * j:128 * (j + 1)],
                        rhs=ct[k][:], start=(k == 0), stop=False))
                for k in range(4):
                    mms.append(nc.tensor.matmul(
                        ps[:], lhsT=wfv[k][:, 128 * j:128 * (j + 1)],
                        rhs=ct[4 + k][:], start=False, stop=(k == 3)))
                dst = phid.tile([128, 384], BF, tag="hid")
                if has_c1:
                    nc.scalar.activation(dst[:], ps[:], AF.Relu,
                                         bias=bia[:, 24 + j:24 + j + 1])
                else:
                    nc.scalar.activation(dst[:], ps[:], AF.Relu)
                hid.append(dst)
            for j in range(4):
                ps = psA.tile([128, 384], F32, tag="proj",
                              padded_shape=[128, 512])
                for k in range(4):
                    nc.tensor.matmul(ps[:],
                                     lhsT=wm2[k][:, 128 * j:128 * (j + 1)],
                                     rhs=hid[k][:],
                                     start=(k == 0), stop=(k == 3))
                osb = pout.tile([128, 384], F32, tag="o")
                if has_b2:
                    nc.scalar.activation(osb[:], ps[:], AF.Identity,
                                         bias=bia[:, 28 + j:28 + j + 1])
                else:
                    nc.scalar.activation(osb[:], ps[:], AF.Copy)
                nc.sync.dma_start(out_t[128 * j:128 * (j + 1), g0:g0 + 2 * S],
                                  osb[:])

        # ---- software-pipelined emission ----
        # attention of pair g interleaves with QK projections of pair g+1
        # (dense N=384 matmuls) so the PE array duty stays high and HAM
        # keeps the 2.4 GHz clock.
        emit_dma(0)
        emit_dma(1)
        for g in range(NPAIR):
            ct_all[g] = [pct.tile([128, 2 * S], BF, tag="ct",
                                  name=f"ct_{g}_{i}") for i in range(8)]
            if g == 0:
                emit_qk(0, "h")
                emit_qk(0, "v")
            if g + 2 < NPAIR:
                emit_dma(g + 2)
            for a in range(2):
                emit_attn_sb(g, a, "h")
                emit_attn_sb(g, a, "v")
                if g + 1 < NPAIR:
                    emit_qk(g + 1, "h" if a == 0 else "v")
                # MLP of the PREVIOUS pair: its ctx evictions are long done,
                # so these dense matmuls never stall the PE stream.
                if a == 1 and g > 0:
                    emit_mlp(g - 1)
        emit_mlp(NPAIR - 1)
    nc.finalize()
    return nc


_CACHE = {}


def _get_program(bias_flags):
    key = tuple(bias_flags)
    if key not in _CACHE:
        _CACHE[key] = _build_program(key)
    return _CACHE[key]


def _col(b):
    """bias vector (128*n,) -> (128, n) column-pack, fortran-ish layout."""
    return np.ascontiguousarray(b.reshape(-1, 128).T.astype(np.float32))


def kernel(hidden_states, h_in_w, h_in_b, h_out_w, h_out_b,
           v_in_w, v_in_b, v_out_w, v_out_b,
           mlp_w1, mlp_b1, mlp_w2, mlp_b2):
    x = np.asarray(hidden_states, dtype=np.float32)
    h_in_w = np.asarray(h_in_w, np.float32)
    h_in_b = np.asarray(h_in_b, np.float32)
    h_out_w = np.asarray(h_out_w, np.float32)
    h_out_b = np.asarray(h_out_b, np.float32)
    v_in_w = np.asarray(v_in_w, np.float32)
    v_in_b = np.asarray(v_in_b, np.float32)
    v_out_w = np.asarray(v_out_w, np.float32)
    v_out_b = np.asarray(v_out_b, np.float32)
    mlp_w1 = np.asarray(mlp_w1, np.float32)
    mlp_b1 = np.asarray(mlp_b1, np.float32)
    mlp_w2 = np.asarray(mlp_w2, np.float32)
    mlp_b2 = np.asarray(mlp_b2, np.float32)

    # V biases act as a constant shift of ctx (softmax weights sum to 1),
    # so fold them through the out-projections; then fold the
    # out-projections themselves into MLP1 (everything is linear up to the
    # ReLU): hid = relu(W1h(Who ctx_h + bho) + W1v(Wvo ctx_v + bvo) + b1)
    #            = relu(F_h ctx_h + F_v ctx_v + c1)
    h_out_eff = h_out_b + h_out_w @ h_in_b[2 * E:3 * E]
    v_out_eff = v_out_b + v_out_w @ v_in_b[2 * E:3 * E]
    w1h = mlp_w1[:, 0:E]
    w1v = mlp_w1[:, E:2 * E]
    f_h = w1h @ h_out_w          # (E, E): hid += F_h @ ctx_h
    f_v = w1v @ v_out_w
    c1 = w1h @ h_out_eff + w1v @ v_out_eff + mlp_b1

    bias_flags = (
        bool(np.any(v_in_b[0:2 * E])), bool(np.any(h_in_b[0:E])),
        bool(np.any(h_in_b[E:2 * E])), bool(np.any(c1)), bool(np.any(mlp_b2)),
    )
    nc = _get_program(bias_flags)

    biases = np.zeros((128, 32), np.float32)
    biases[:, 0:8] = _col(v_in_b[0:2 * E])
    biases[:, 8:16] = _col(h_in_b[0:2 * E])
    biases[:, 24:28] = _col(c1)
    biases[:, 28:32] = _col(mlp_b2)

    shared = {
        "w_vin": np.ascontiguousarray(v_in_w.T).astype(NPBF),
        "w_hq": np.ascontiguousarray(h_in_w[0:E].T).astype(NPBF),
        "w_hkv": np.ascontiguousarray(h_in_w[E:3 * E].T).astype(NPBF),
        "w_fh": np.ascontiguousarray(f_h.T).astype(NPBF),
        "w_fv": np.ascontiguousarray(f_v.T).astype(NPBF),
        "w_m2": np.ascontiguousarray(mlp_w2.T).astype(NPBF),
        "mask": _band_masks(),
        "biases": biases,
    }

    in_maps = []
    for c in range(NCORE):
        rows = x[RPC * c:RPC * (c + 1)]                      # (24, 192, 512)
        cols = x[:, RPC * c:RPC * (c + 1)].transpose(1, 0, 2)  # (24, 192, 512)
        m = dict(shared)
        m["xr_t"] = np.ascontiguousarray(rows.reshape(T, E).T).astype(NPBF)
        m["xc_t"] = np.ascontiguousarray(cols.reshape(T, E).T).astype(NPBF)
        in_maps.append(m)

    global _LAST_IN_MAPS
    _LAST_IN_MAPS = in_maps
    res = run_bass_kernel_spmd(nc, in_maps, core_ids=list(range(NCORE)))

    out = np.empty((S, S, E), np.float32)
    for c in range(NCORE):
        out[RPC * c:RPC * (c + 1)] = res.results[c]["out_t"].T.reshape(RPC, S, E)
    return out


# revision 20
# speedup vs baseline: 1.5241x; 1.2041x over previous
"""BiSPA (bidirectional sparse windowed attention + MLP) Trainium2 kernel.

Full inputs in, full outputs out. Internally shards across 8 NeuronCores:
core c owns output rows Ic = [24c, 24c+24) of the (192, 192, 512) grid.

Key observation: with B == S == 192 and window W == 32,
  - vertical attention for output row i is a complete 192-token sliding-window
    attention over x[i, :, :]                        (needs x rows  Ic)
  - horizontal attention for output row i is a complete 192-token
    sliding-window attention with Q from x[i, :, :] and K/V from x[:, i, :]
    (needs x columns Ic)
so each core needs x[Ic, :, :] and x[:, Ic, :] and NOTHING else -> zero
duplicated projection FLOPs, zero collectives, no halos.

v2 changes vs the first working kernel (tensor-engine bound at ~864us):
  - out-projections folded into MLP1 on the host: F_h = W1h @ Who,
    F_v = W1v @ Wvo, c1 = W1h@h_out_eff + W1v@v_out_eff + b1. Removes
    384 N=384 matmuls + 8 evictions per pair.
  - scores for a head pair land in one 2-bank PSUM tile -> ONE exp
    activation per head pair (batched [128, 2, 224] strided read).
  - softmax normalization: one batched reciprocal per head pair
    ([128,2,2,1] AP over the 4 Z columns), 4 DVE scale ops.
  - ctx^T transposes stay on the PE but write f32 into SPARE COLUMNS
    (260:452) of the same PSUM bank as the attn@V accumulator, so no
    extra PSUM banks are needed; ONE ACT copy per head pair evicts.
  - emission software-pipelined: QK projections of pair g+1 interleave
    with the attention inner loop of pair g so the PE array duty never
    dips long enough for HAM to re-throttle the clock (354us of the
    baseline ran at 1.2 GHz instead of 2.4); MLP of pair g is emitted
    during pair g+1 so it never waits on fresh ctx evictions.

Numerics: all matmul inputs bf16, fp32 PSUM accumulation, fp32 softmax exp
input, bf16 probs/ctx.  Measured end-to-end rel err vs fp32 reference ~0.6%.
"""

import numpy as np
from contextlib import ExitStack

import concourse.bass as bass
import concourse.mybir as mybir
import concourse.tile as tile
from concourse import bacc
from concourse.bass_utils import run_bass_kernel_spmd
from concourse.masks import make_identity
from concourse.tile import add_dep_helper


def _chain(insts):
    """Order matmuls targeting one PSUM bank: a start=True zeroes (marks
    pending-zero) the WHOLE 2KB bank, so each bank must hold exactly one
    accumulation group and the group's matmuls must execute in program order.
    Tile won't order disjoint-region writes by itself."""
    for a, b in zip(insts, insts[1:]):
        add_dep_helper(b.ins, a.ins, sync=False, reason="psum-bank group order")

BF = mybir.dt.bfloat16
F32 = mybir.dt.float32
AF = mybir.ActivationFunctionType
MUL = mybir.AluOpType.mult
NPBF = mybir.dt.np(BF)

E = 512
H = 8
D = 64
W = 32
S = 192
NCORE = 8
RPC = 24          # rows (strips) per core
T = RPC * S       # tokens per core per branch = 4608
NPAIR = RPC // 2  # strip pairs per core


def _band_masks():
    """Score mask, bf16 (128, 384): [TA 96 | TB 96] x 2 heads.

    TA: rows p = key k in [0,128), cols q in [0,96).
        valid = |k-q| <= W              (every key for q<96 is in [0,128))
    TB: rows p -> key k = 64+p in [64,192), cols q' -> q = 96+q' in [96,192).
        valid = |k-q| <= W              (every key for q>=96 is in [64,192))

    q-ranges are disjoint between TA and TB, so the three attn@V matmuls
    per head write DISJOINT PSUM cells (no accumulate-overlap -> the PE
    pipelines them instead of serializing on the drain).
    """
    m = np.zeros((128, 192), np.float32)
    k = np.arange(128)[:, None]
    q = np.arange(96)[None, :]
    m[:, 0:96] = np.abs(k - q) <= W
    kb = 64 + np.arange(128)[:, None]
    qb = 96 + np.arange(96)[None, :]
    m[:, 96:192] = np.abs(kb - qb) <= W
    return np.concatenate([m, m], axis=1).astype(NPBF)


def _build_program(bias_flags):
    """Build the SPMD Bass/Tile program (same program on all 8 cores)."""
    has_vqk_b, has_hq_b, has_hk_b, has_c1, has_b2 = bias_flags

    nc = bacc.Bacc("TRN2", target_bir_lowering=False, debug=False,
                   num_devices=NCORE, num_swdge_queues=4)

    xr_t = nc.dram_tensor("xr_t", [E, T], BF, kind="ExternalInput").ap()
    xc_t = nc.dram_tensor("xc_t", [E, T], BF, kind="ExternalInput").ap()
    w_vin = nc.dram_tensor("w_vin", [E, 3 * E], BF, kind="ExternalInput").ap()
    w_hq = nc.dram_tensor("w_hq", [E, E], BF, kind="ExternalInput").ap()
    w_hkv = nc.dram_tensor("w_hkv", [E, 2 * E], BF, kind="ExternalInput").ap()
    w_fh = nc.dram_tensor("w_fh", [E, E], BF, kind="ExternalInput").ap()
    w_fv = nc.dram_tensor("w_fv", [E, E], BF, kind="ExternalInput").ap()
    w_m2 = nc.dram_tensor("w_m2", [E, E], BF, kind="ExternalInput").ap()
    mask_d = nc.dram_tensor("mask", [128, 384], BF, kind="ExternalInput").ap()
    bias_d = nc.dram_tensor("biases", [128, 32], F32, kind="ExternalInput").ap()
    out_t = nc.dram_tensor("out_t", [E, T], F32, kind="ExternalOutput").ap()

    with tile.TileContext(nc) as tc, ExitStack() as ctx:
        pw = ctx.enter_context(tc.tile_pool(name="pw", bufs=1))
        psA = ctx.enter_context(tc.tile_pool(name="psA", bufs=2, space="PSUM"))
        psS = ctx.enter_context(tc.tile_pool(name="psS", bufs=2, space="PSUM"))
        psC = ctx.enter_context(tc.tile_pool(name="psC", bufs=2, space="PSUM"))
        px = ctx.enter_context(tc.tile_pool(name="px", bufs=3))
        pqk = ctx.enter_context(tc.tile_pool(name="pqk", bufs=32))
        pv = ctx.enter_context(tc.tile_pool(name="pv", bufs=8))
        pp = ctx.enter_context(tc.tile_pool(name="pp", bufs=12))
        pctx = ctx.enter_context(tc.tile_pool(name="pctx", bufs=8))
        pzr = ctx.enter_context(tc.tile_pool(name="pzr", bufs=8))
        pct = ctx.enter_context(tc.tile_pool(name="pct", bufs=16))
        phid = ctx.enter_context(tc.tile_pool(name="phid", bufs=8))
        pout = ctx.enter_context(tc.tile_pool(name="pout", bufs=8))

        # ---- persistent constants ----
        def load_const(name, dram_ap, shape, dtype):
            t = pw.tile(shape, dtype, tag=name)
            nc.gpsimd.dma_start(t[:], dram_ap)
            return t

        wv = [load_const(f"wv{k}", w_vin[128 * k:128 * (k + 1), :], [128, 3 * E], BF)
              for k in range(4)]
        whq = [load_const(f"whq{k}", w_hq[128 * k:128 * (k + 1), :], [128, E], BF)
               for k in range(4)]
        whkv = [load_const(f"whkv{k}", w_hkv[128 * k:128 * (k + 1), :], [128, 2 * E], BF)
                for k in range(4)]
        wfh = [load_const(f"wfh{k}", w_fh[128 * k:128 * (k + 1), :], [128, E], BF)
               for k in range(4)]
        wfv = [load_const(f"wfv{k}", w_fv[128 * k:128 * (k + 1), :], [128, E], BF)
               for k in range(4)]
        wm2 = [load_const(f"wm2{k}", w_m2[128 * k:128 * (k + 1), :], [128, E], BF)
               for k in range(4)]
        msk = load_const("msk", mask_d[:, :], [128, 384], BF)
        bia = load_const("bia", bias_d[:, :], [128, 32], F32)
        ident = pw.tile([128, 128], BF, tag="ident")
        make_identity(nc, ident)

        # bias column map (within `bia`):
        # 0-7 v_in_b[0:1024] ftiles; 8-11 h_in_b[0:512]; 12-15 h_in_b[512:1024]
        # 24-27 c1 (fused W1@out_bias + mlp_b1); 28-31 mlp_b2

        # per-pair state, filled by the emit helpers
        xr2_all = [None] * NPAIR
        xc2_all = [None] * NPAIR
        qk_all = [None] * NPAIR
        ct_all = [None] * NPAIR
        # ACT evictions of past cp tiles (psC bufs=2): the bitcast APs used
        # by the transposes/evict may not be seen by Tile's overlap tracker,
        # so the next occupant of the same PSUM buffer syncs explicitly.
        cp_evicts = []

        def emit_dma(g):
            g0 = 2 * S * g
            xr2, xc2 = [], []
            for k in range(4):
                t = px.tile([128, 2 * S], BF, tag=f"xr{k}")
                nc.gpsimd.dma_start(t[:], xr_t[128 * k:128 * (k + 1), g0:g0 + 2 * S])
                xr2.append(t)
                t = px.tile([128, 2 * S], BF, tag=f"xc{k}")
                nc.gpsimd.dma_start(t[:], xc_t[128 * k:128 * (k + 1), g0:g0 + 2 * S])
                xc2.append(t)
            xr2_all[g] = xr2
            xc2_all[g] = xc2

        def emit_qk(g, branch):
            """QK projections for one branch of pair g, feature-major, N=384.
            ftile j in 0..7: j<4 -> Q features, j>=4 -> K features."""
            if qk_all[g] is None:
                qk_all[g] = {}
            xr2, xc2 = xr2_all[g], xc2_all[g]
            qk_all[g][branch] = []
            for j in range(8):
                ps = psA.tile([128, 384], F32, tag="proj",
                              padded_shape=[128, 512])
                for k in range(4):
                    if branch == "v":
                        lhsT = wv[k][:, 128 * j:128 * (j + 1)]
                        rhs = xr2[k][:]
                    elif j < 4:   # h Q
                        lhsT = whq[k][:, 128 * j:128 * (j + 1)]
                        rhs = xr2[k][:]
                    else:         # h K
                        lhsT = whkv[k][:, 128 * (j - 4):128 * (j - 3)]
                        rhs = xc2[k][:]
                    nc.tensor.matmul(ps[:], lhsT=lhsT, rhs=rhs,
                                     start=(k == 0), stop=(k == 3))
                # bias columns: v ftiles 0-7 -> cols 0-7; h Q 0-3 -> 8-11;
                # h K 0-3 -> 12-15
                dst = pqk.tile([128, 384], BF, tag="qk")
                need_b = (has_vqk_b if branch == "v"
                          else (has_hq_b if j < 4 else has_hk_b))
                if need_b:
                    bcol = j if branch == "v" else (8 + j)
                    nc.scalar.activation(dst[:], ps[:], AF.Identity,
                                         bias=bia[:, bcol:bcol + 1])
                else:
                    nc.scalar.activation(dst[:], ps[:], AF.Copy)
                qk_all[g][branch].append(dst)

        def emit_attn_sb(g, a, br):
            """Attention for strip a (0/1) of pair g, branch br."""
            s0 = S * a
            xin = xr2_all[g] if br == "v" else xc2_all[g]
            vcols = slice(1024, 1536) if br == "v" else slice(512, 1024)
            vw = wv if br == "v" else whkv
            qk = qk_all[g][br]
            ct = ct_all[g]

            # ------ V projection, token-major, with ones column ----
            vps_a = psA.tile([128, 512], F32, tag="proj")
            vps_b = psA.tile([128, 512], F32, tag="proj")
            for k in range(4):
                nc.tensor.matmul(vps_a[:], lhsT=xin[k][:, s0:s0 + 128],
                                 rhs=vw[k][:, vcols],
                                 start=(k == 0), stop=(k == 3))
            for k in range(4):
                nc.tensor.matmul(vps_b[:], lhsT=xin[k][:, s0 + 64:s0 + 192],
                                 rhs=vw[k][:, vcols],
                                 start=(k == 0), stop=(k == 3))
            va = pv.tile([128, 8, 65], BF, tag="vp")   # keys [0:128)
            vb = pv.tile([128, 8, 65], BF, tag="vp")   # keys [64:192)
            nc.vector.tensor_copy(
                va[:, :, 0:64],
                vps_a[:].rearrange("p (h c) -> p h c", c=64))
            nc.vector.tensor_copy(
                vb[:, :, 0:64],
                vps_b[:].rearrange("p (h c) -> p h c", c=64))
            nc.vector.memset(va[:, :, 64:65], 1.0)
            nc.vector.memset(vb[:, :, 64:65], 1.0)

            # ------ scores + exp + mask for all 4 head pairs ------
            pms = []
            for p in range(4):
                QT = qk[p][:, s0:s0 + S]
                KT = qk[4 + p][:, s0:s0 + S]
                # one 2-bank tile per head pair; head h2 -> bank h2.
                # Matmuls with disjoint contraction row-groups (head0 at
                # partitions 0:64, head1 at 64:128) run CONCURRENTLY on the
                # PE and hard-fault if they write the same PSUM bank.
                sp = psS.tile([128, 2, 512], F32, tag="sc")
                for h2 in range(2):
                    d0 = 64 * h2
                    nc.tensor.matmul(sp[:, h2:h2 + 1, 0:96],
                                     lhsT=KT[d0:d0 + 64, 0:128],
                                     rhs=QT[d0:d0 + 64, 0:96],
                                     start=True, stop=True)
                    nc.tensor.matmul(sp[:, h2:h2 + 1, 96:192],
                                     lhsT=KT[d0:d0 + 64, 64:192],
                                     rhs=QT[d0:d0 + 64, 96:192],
                                     start=True, stop=True)
                # ONE exp for both heads: strided [128, 2, 192] PSUM read
                pb = pp.tile([128, 512], BF, tag="p")
                nc.scalar.activation(
                    pb[:, 0:384].rearrange("p (b c) -> p b c", b=2),
                    sp[:, :, 0:192], AF.Exp, scale=0.125)
                pm = pp.tile([128, 512], BF, tag="p")
                nc.vector.tensor_tensor(pm[:, 0:384], pb[:, 0:384],
                                        msk[:, 0:384], op=MUL)
                pms.append(pm)

            # ------ attn@V + normalize + ctx transpose per head pair ------
            for p in range(4):
                pm = pms[p]
                # attn@V, one accumulation "group" per bank, ordered by
                # _chain. All matmuls use base-0 contraction rows padded to
                # overlapping ranges (the pad rows are band-mask zeros in pm),
                # so none of them can run concurrently and collide on the
                # bank. The built-in group checker cannot express
                # multi-region banks, so skip it; correctness comes from the
                # explicit ordering + per-element pending-zero semantics.
                cp = psC.tile([128, 512], F32, tag="cx")
                mms = []
                for h2 in range(2):
                    h = 2 * p + h2
                    cb = 130 * h2
                    ta = 192 * h2
                    tb = 192 * h2 + 96
                    # q in [0,96): keys [0:128) from TA
                    mms.append(nc.tensor.matmul(
                        cp[0:96, cb:cb + 65], lhsT=pm[:, ta:ta + 96],
                        rhs=va[:, h:h + 1, :], start=(h2 == 0),
                        stop=False, skip_group_check=True))
                    # q in [96,128): keys [64:160) = TB rows [0:96).
                    # start=True here too: mm1 only spans partitions 0:96,
                    # so rows 96:128 of a FRESH bank would keep stale
                    # has_written bits and this mm would accumulate onto
                    # power-on garbage. Every cell has exactly one writer,
                    # so an extra pending-zero can never corrupt values.
                    mms.append(nc.tensor.matmul(
                        cp[96:128, cb:cb + 65],
                        lhsT=pm[0:96, tb:tb + 32],
                        rhs=vb[0:96, h:h + 1, :],
                        start=(h2 == 0), stop=False, tile_position=(0, 96),
                        skip_group_check=True))
                    # q in [128,192): keys [96:192) = TB rows [32:128),
                    # K padded to rows [0:128) (rows 0:32 masked 0)
                    mms.append(nc.tensor.matmul(
                        cp[0:64, cb + 65:cb + 130],
                        lhsT=pm[0:128, tb + 32:tb + 96],
                        rhs=vb[0:128, h:h + 1, :],
                        start=False, stop=(h2 == 1),
                        skip_group_check=True))
                _chain(mms)
                if len(cp_evicts) >= 2:
                    add_dep_helper(mms[0].ins, cp_evicts[-2].ins, sync=True,
                                   reason="cp buffer reuse after evict")

                # normalize by 1/Z (Z = ones-column accumulation at cols
                # {64, 129, 194, 259} = 64 + 130h + 65q) and pack for the
                # transpose: ctxn = [h0q1 | h1q1 | h0q2 | h1q2], 64 cols each
                zr = pzr.tile([128, 2, 2, 1], F32, tag="zr")
                zin = (cp[:, 64:324]
                       .rearrange("p (h x) -> p h x", h=2)
                       .rearrange("p h (q c) -> p h q c", q=2))
                ctxn = pctx.tile([128, 256], BF, tag="ctxn")
                # single tensor_tensor: ctx blocks (h, q) x broadcast 1/Z.
                # q2 blocks only have 64 valid partitions; rows 64:128 of
                # those blocks compute garbage (stale psum x recip(stale))
                # that the q2 transpose never reads.
                cpq = (cp[:, 0:260]
                       .rearrange("p (h x) -> p h x", h=2)
                       .rearrange("p h (q c) -> p h q c", q=2))
                cto = ctxn[:].rearrange("p (q h c) -> p h q c", q=2, h=2)
                reads = [
                    nc.vector.reciprocal(zr[:], zin[:, :, :, 0:1]),
                    nc.vector.tensor_tensor(
                        cto, cpq[:, :, :, 0:64],
                        zr[:, :, :, 0:1].broadcast_to([128, 2, 2, 64]),
                        op=MUL),
                ]
                # cp reads must wait for the accumulation group to close
                # (same-bank PE-write + DVE-read is a HW fault)
                for r in reads:
                    add_dep_helper(r.ins, mms[-1].ins, sync=True,
                                   reason="psum read after group close")

                # ctx^T via PE transpose, writing f32 into the SPARE columns
                # (260:452) of the same PSUM bank as cp -- no extra banks.
                # The transposes must not overlap the DVE normalize reads of
                # this bank (PE-write + DVE-read same bank is a HW fault).
                tps = [
                    nc.tensor.transpose(cp[:, 260:324].bitcast(BF),
                                        ctxn[:, 0:128], ident[:]),
                    nc.tensor.transpose(cp[:, 324:356].bitcast(BF),
                                        ctxn[0:64, 128:256],
                                        ident[0:64, 0:64]),
                ]
                for t in tps:
                    for r in reads:
                        add_dep_helper(t.ins, r.ins, sync=True,
                                       reason="transpose after bank reads")
                ct_p = ct[(0 if br == "h" else 4) + p]
                ev = nc.scalar.activation(ct_p[:, s0:s0 + S],
                                          cp[:, 260:356].bitcast(BF), AF.Copy)
                for t in tps:
                    add_dep_helper(ev.ins, t.ins, sync=True,
                                   reason="evict after transpose")
                cp_evicts.append(ev)

        def emit_mlp(g):
            """Fused (out-proj + MLP1) then MLP2 for pair g, N=384."""
            ct = ct_all[g]
            g0 = 2 * S * g
            hid = []
            for j in range(4):
                ps = psA.tile([128, 384], F32, tag="proj",
                              padded_shape=[128, 512])
                mms = []
                for k in range(4):
                    mms.append(nc.tensor.matmul(
                        ps[:], lhsT=wfh[k][:, 128 * j:128 * (j + 1)],
                        rhs=ct[k][:], start=(k == 0), stop=False))
                for k in range(4):
                    mms.append(nc.tensor.matmul(
                        ps[:], lhsT=wfv[k][:, 128 * j:128 * (j + 1)],
                        rhs=ct[4 + k][:], start=False, stop=(k == 3)))
                dst = phid.tile([128, 384], BF, tag="hid")
                if has_c1:
                    nc.scalar.activation(dst[:], ps[:], AF.Relu,
                                         bias=bia[:, 24 + j:24 + j + 1])
                else:
                    nc.scalar.activation(dst[:], ps[:], AF.Relu)
                hid.append(dst)
            for j in range(4):
                ps = psA.tile([128, 384], F32, tag="proj",
                              padded_shape=[128, 512])
                for k in range(4):
                    nc.tensor.matmul(ps[:],
                                     lhsT=wm2[k][:, 128 * j:128 * (j + 1)],
                                     rhs=hid[k][:],
                                     start=(k == 0), stop=(k == 3))
                osb = pout.tile([128, 384], F32, tag="o")
                if has_b2:
                    nc.scalar.activation(osb[:], ps[:], AF.Identity,
                                         bias=bia[:, 28 + j:28 + j + 1])
                else:
                    nc.scalar.activation(osb[:], ps[:], AF.Copy)
                nc.sync.dma_start(out_t[128 * j:128 * (j + 1), g0:g0 + 2 * S],
                                  osb[:])

        # ---- software-pipelined emission ----
        # attention of pair g interleaves with QK projections of pair g+1
        # (dense N=384 matmuls) so the PE array duty stays high and HAM
        # keeps the 2.4 GHz clock.
        emit_dma(0)
        emit_dma(1)
        for g in range(NPAIR):
            ct_all[g] = [pct.tile([128, 2 * S], BF, tag="ct",
                                  name=f"ct_{g}_{i}") for i in range(8)]
            if g == 0:
                emit_qk(0, "h")
                emit_qk(0, "v")
            if g + 2 < NPAIR:
                emit_dma(g + 2)
            for a in range(2):
                emit_attn_sb(g, a, "h")
                emit_attn_sb(g, a, "v")
                if g + 1 < NPAIR:
                    emit_qk(g + 1, "h" if a == 0 else "v")
                # MLP of the PREVIOUS pair: its ctx evictions are long done,
                # so these dense matmuls never stall the PE stream.
                if a == 1 and g > 0:
                    emit_mlp(g - 1)
        emit_mlp(NPAIR - 1)
    nc.finalize()
    return nc


_CACHE = {}


def _get_program(bias_flags):
    key = tuple(bias_flags)
    if key not in _CACHE:
        _CACHE[key] = _build_program(key)
    return _CACHE[key]


def _col(b):
    """bias vector (128*n,) -> (128, n) column-pack, fortran-ish layout."""
    return np.ascontiguousarray(b.reshape(-1, 128).T.astype(np.float32))


def kernel(hidden_states, h_in_w, h_in_b, h_out_w, h_out_b,
           v_in_w, v_in_b, v_out_w, v_out_b,
           mlp_w1, mlp_b1, mlp_w2, mlp_b2):
    x = np.asarray(hidden_states, dtype=np.float32)
    h_in_w = np.asarray(h_in_w, np.float32)
    h_in_b = np.asarray(h_in_b, np.float32)
    h_out_w = np.asarray(h_out_w, np.float32)
    h_out_b = np.asarray(h_out_b, np.float32)
    v_in_w = np.asarray(v_in_w, np.float32)
    v_in_b = np.asarray(v_in_b, np.float32)
    v_out_w = np.asarray(v_out_w, np.float32)
    v_out_b = np.asarray(v_out_b, np.float32)
    mlp_w1 = np.asarray(mlp_w1, np.float32)
    mlp_b1 = np.asarray(mlp_b1, np.float32)
    mlp_w2 = np.asarray(mlp_w2, np.float32)
    mlp_b2 = np.asarray(mlp_b2, np.float32)

    # V biases act as a constant shift of ctx (softmax weights sum to 1),
    # so fold them through the out-projections; then fold the
    # out-projections themselves into MLP1 (everything is linear up to the
    # ReLU): hid = relu(W1h(Who ctx_h + bho) + W1v(Wvo ctx_v + bvo) + b1)
    #            = relu(F_h ctx_h + F_v ctx_v + c1)
    h_out_eff = h_out_b + h_out_w @ h_in_b[2 * E:3 * E]
    v_out_eff = v_out_b + v_out_w @ v_in_b[2 * E:3 * E]
    w1h = mlp_w1[:, 0:E]
    w1v = mlp_w1[:, E:2 * E]
    f_h = w1h @ h_out_w          # (E, E): hid += F_h @ ctx_h
    f_v = w1v @ v_out_w
    c1 = w1h @ h_out_eff + w1v @ v_out_eff + mlp_b1

    bias_flags = (
        bool(np.any(v_in_b[0:2 * E])), bool(np.any(h_in_b[0:E])),
        bool(np.any(h_in_b[E:2 * E])), bool(np.any(c1)), bool(np.any(mlp_b2)),
    )
    nc = _get_program(bias_flags)

    biases = np.zeros((128, 32), np.float32)
    biases[:, 0:8] = _col(v_in_b[0:2 * E])
    biases[:, 8:16] = _col(h_in_b[0:2 * E])
    biases[:, 24:28] = _col(c1)
    biases[:, 28:32] = _col(mlp_b2)

    shared = {
        "w_vin": np.ascontiguousarray(v_in_w.T).astype(NPBF),
        "w_hq": np.ascontiguousarray(h_in_w[0:E].T).astype(NPBF),
        "w_hkv": np.ascontiguousarray(h_in_w[E:3 * E].T).astype(NPBF),
        "w_fh": np.ascontiguousarray(f_h.T).astype(NPBF),
        "w_fv": np.ascontiguousarray(f_v.T).astype(NPBF),
        "w_m2": np.ascontiguousarray(mlp_w2.T).astype(NPBF),
        "mask": _band_masks(),
        "biases": biases,
    }

    in_maps = []
    for c in range(NCORE):
        rows = x[RPC * c:RPC * (c + 1)]                      # (24, 192, 512)
        cols = x[:, RPC * c:RPC * (c + 1)].transpose(1, 0, 2)  # (24, 192, 512)
        m = dict(shared)
        m["xr_t"] = np.ascontiguousarray(rows.reshape(T, E).T).astype(NPBF)
        m["xc_t"] = np.ascontiguousarray(cols.reshape(T, E).T).astype(NPBF)
        in_maps.append(m)

    global _LAST_IN_MAPS
    _LAST_IN_MAPS = in_maps
    res = run_bass_kernel_spmd(nc, in_maps, core_ids=list(range(NCORE)))

    out = np.empty((S, S, E), np.float32)
    for c in range(NCORE):
        out[RPC * c:RPC * (c + 1)] = res.results[c]["out_t"].T.reshape(RPC, S, E)
    return out
